# revision 7
# baseline (speedup 1.0000x reference)
"""CondConv2d Trainium2 kernel.

B=32, C=192, H=W=64, O=192, E=8, 3x3 'same' conv.
Data-parallel over batch: 8 cores x 4 samples. Expert weights replicated.

Per-core pipeline:
  Phase A: stream x, per-(sample,channel) sums (global avg pool numerator).
  Routing: logits = sums @ routing_w.T; sigmoid(logits/4096 + b) on ACT.
  Phase B: mix expert weights with a block-diagonal PE matmul
           (K = 16 o-groups x 8 experts = 128 -> 16x fewer streamed columns
           than the naive K=8 mixing), partition-remap via SBUF->SBUF DMA,
           then PE transposes to per-sample lhsT[c, (kk,o)] layout.
  Phase C: conv = 9 shifted float32r matmuls accumulated in PSUM per
           8-row output strip; evict via DVE; DMA out.
"""

import sys
import numpy as np

for _p in ("/opt/trn_rl_repo",):
    if _p not in sys.path:
        sys.path.insert(0, _p)

BS = 4          # samples per core
C = 192
H = W = 64
O = 192
E = 8
KK = 9          # 3x3
CKK = C * KK    # 1728, per-o flattened (c,kh,kw) block in expert_weight rows
N_CORES = 8

# mixing pass structure: o-groups of 8 o-values
OSUB = 8
G1 = 16         # pass 1: o in [0,128)
G2 = 8          # pass 2: o in [128,192)
GLEN = OSUB * CKK          # 13824 elements of an expert row per group
MIXN = 432                 # mixing matmul free dim (>=256 keeps f32r at 1 cyc/col)
T_PER_G = GLEN // MIXN     # 32

_COMPILED = None


def _build():
    import concourse.bass as bass
    import concourse.bacc as bacc
    import concourse.mybir as mybir
    import concourse.tile as tile
    from concourse import masks

    f32 = mybir.dt.float32
    f32r = mybir.dt.float32r
    AX = mybir.AxisListType
    ACT = mybir.ActivationFunctionType

    nc = bacc.Bacc("TRN2", target_bir_lowering=False, debug=False)

    x_d = nc.dram_tensor("x", [BS, C, H, W], f32, kind="ExternalInput")
    x_r = nc.dram_tensor("x_r", [BS, C, H, W], f32r, kind="ExternalInput")
    ew_d = nc.dram_tensor("expert_weight", [E, O * CKK], f32r, kind="ExternalInput")
    rw_d = nc.dram_tensor("routing_w", [E, C], f32, kind="ExternalInput")
    rb_d = nc.dram_tensor("routing_b", [E], f32, kind="ExternalInput")
    zp_d = nc.dram_tensor("zpad", [128, 128], f32r, kind="ExternalInput")
    out_d = nc.dram_tensor("out", [BS, O, H, W], f32, kind="ExternalOutput")

    with tile.TileContext(nc) as tc:
        with (
            tc.tile_pool(name="persist", bufs=1) as persist,
            tc.tile_pool(name="psum_small", bufs=1, space="PSUM") as psum_small,
        ):
            # ---------- persistent small tiles ----------
            ident = persist.tile([128, 128], f32)
            masks.make_identity(nc, ident[:])

            rwT_lo = persist.tile([128, E], f32)
            rwT_hi = persist.tile([64, E], f32)
            rwT_src = rw_d.ap().rearrange("e c -> c e")
            nc.sync.dma_start(rwT_lo[:], rwT_src[0:128])
            nc.sync.dma_start(rwT_hi[:], rwT_src[128:192])
            rb_t = persist.tile([E, 1], f32)
            nc.sync.dma_start(rb_t[:], rb_d.ap().unsqueeze(1))

            pooled_lo = persist.tile([128, BS], f32)
            pooled_hi = persist.tile([64, BS], f32)

            # per-sample conv weights, transposed layout [c, kk*192+o]
            lhsT_lo = [persist.tile([128, KK * O], f32r, name=f"lhsT_lo{b}") for b in range(BS)]
            lhsT_hi = [persist.tile([64, KK * O], f32r, name=f"lhsT_hi{b}") for b in range(BS)]

            # ---------- Phase A: x sums for global avg pool ----------
            with tc.tile_pool(name="apool", bufs=2) as apool:
                for s in range(BS):
                    xa_lo = apool.tile([128, H, W], f32, tag="xa_lo")
                    nc.sync.dma_start(xa_lo[:], x_d[s, 0:128])
                    nc.vector.reduce_sum(pooled_lo[:, s : s + 1], xa_lo[:], axis=AX.XY)
                    xa_hi = apool.tile([64, H, W], f32, tag="xa_hi")
                    nc.sync.dma_start(xa_hi[:], x_d[s, 128:192])
                    nc.vector.reduce_sum(pooled_hi[:, s : s + 1], xa_hi[:], axis=AX.XY)

            # ---------- routing ----------
            psum_r = psum_small.tile([E, BS], f32)
            nc.tensor.matmul(psum_r[:], rwT_lo[:], pooled_lo[:], start=True, stop=False)
            nc.tensor.matmul(psum_r[:], rwT_hi[:], pooled_hi[:], start=False, stop=True)
            r_sb = persist.tile([E, BS], f32r)
            nc.scalar.activation(
                r_sb[:], psum_r[:], ACT.Sigmoid, bias=rb_t[:], scale=1.0 / (H * W)
            )

            # block-diagonal mixing weights bd[(g,e), (g,b)] = r[b,e]
            bd1 = persist.tile([G1 * E, G1 * BS], f32r)
            bd2 = persist.tile([G2 * E, G2 * BS], f32r)
            nc.sync.dma_start(bd1[:], zp_d.ap()[: G1 * E, : G1 * BS])
            nc.sync.dma_start(bd2[:], zp_d.ap()[: G2 * E, : G2 * BS])
            for g in range(G1):
                nc.sync.dma_start(
                    bd1[g * E : (g + 1) * E, g * BS : (g + 1) * BS], r_sb[:]
                )
            for g in range(G2):
                nc.sync.dma_start(
                    bd2[g * E : (g + 1) * E, g * BS : (g + 1) * BS], r_sb[:]
                )

            # ---------- Phase B: mix + remap + transpose ----------
            with (
                tc.tile_pool(name="spool", bufs=1) as spool,
                tc.tile_pool(name="natpool", bufs=1) as natpool,
                tc.tile_pool(name="epool", bufs=4) as epool,
                tc.tile_pool(name="mixpsum", bufs=3, space="PSUM") as mixpsum,
                tc.tile_pool(name="tpsum", bufs=3, space="PSUM") as tpsum,
            ):
                nat_lo = [natpool.tile([128, CKK], f32, name=f"nat_lo{b}") for b in range(BS)]
                nat_hi = [natpool.tile([64, CKK], f32, name=f"nat_hi{b}") for b in range(BS)]

                for pidx, (G, bd, nat, o_base) in enumerate(
                    ((G1, bd1, nat_lo, 0), (G2, bd2, nat_hi, 128))
                ):
                    S_all = spool.tile([G * BS, GLEN], f32, tag="S_all", name=f"S{pidx}")
                    ew_view = ew_d.ap()[:, o_base * CKK : (o_base + G * OSUB) * CKK]
                    ew_view = ew_view.rearrange("e (g u) -> g e u", g=G)
                    for t in range(T_PER_G):
                        ewt = epool.tile([G * E, MIXN], f32r, tag="ewt")
                        nc.sync.dma_start(ewt[:], ew_view[:, :, t * MIXN : (t + 1) * MIXN])
                        pm = mixpsum.tile([G * BS, MIXN], f32, tag="pm")
                        nc.tensor.matmul(
                            pm[:], bd[:], ewt[:],
                            start=True, stop=True,
                        )
                        nc.vector.tensor_copy(S_all[:, t * MIXN : (t + 1) * MIXN], pm[:])
                    # partition remap: row (g*BS+b) cols (o_sub,c,kk) -> nat[b][o, (c,kk)]
                    for b in range(BS):
                        for g in range(G):
                            nc.sync.dma_start(
                                nat[b][g * OSUB : (g + 1) * OSUB, :],
                                S_all[g * BS + b : g * BS + b + 1, :].rearrange(
                                    "p (o u) -> p o u", o=OSUB
                                ),
                            )

                # transposes: nat[b][o, (c,kk)] -> lhsT[b][c, (kk,o)]
                for b in range(BS):
                    for kk in range(KK):
                        for cc, (c0, c_n, lhsT) in enumerate(
                            ((0, 128, lhsT_lo[b]), (128, 64, lhsT_hi[b]))
                        ):
                            for oc, (o0, o_n, nat) in enumerate(
                                ((0, 128, nat_lo[b]), (128, 64, nat_hi[b]))
                            ):
                                src = nat[:].rearrange("o (c k) -> o c k", k=KK)[
                                    :, c0 : c0 + c_n, kk
                                ]
                                tp = tpsum.tile([128, 128], f32, tag="tp")
                                nc.tensor.transpose(
                                    tp[:c_n, :o_n], src, ident[:o_n, :o_n]
                                )
                                nc.scalar.copy(
                                    lhsT[:c_n, kk * O + o0 : kk * O + o0 + o_n],
                                    tp[:c_n, :o_n],
                                )

            # ---------- Phase C: conv ----------
            NS = 8          # row strips per sample
            SR = H // NS    # 8 output rows per strip
            with (
                tc.tile_pool(name="cpool", bufs=3) as cpool,
                tc.tile_pool(name="stgpool", bufs=3) as stgpool,
                tc.tile_pool(name="cpsum", bufs=2, space="PSUM") as cpsum,
            ):
                for s in range(BS):
                    for u in range(NS):
                        h0 = u * SR
                        strips = []
                        for cc, (c0, c_n) in enumerate(((0, 128), (128, 64))):
                            st = cpool.tile([c_n, SR + 2, W + 2], f32r, tag=f"strip{cc}")
                            nc.sync.dma_start(st[:, :, 0:1], zp_d.ap()[:c_n, 0 : SR + 2])
                            nc.sync.dma_start(st[:, :, W + 1 : W + 2], zp_d.ap()[:c_n, 0 : SR + 2])
                            if u == 0:
                                nc.sync.dma_start(st[:, 0:1, 1 : W + 1], zp_d.ap()[:c_n, 0:W])
                                nc.sync.dma_start(
                                    st[:, 1 : SR + 2, 1 : W + 1],
                                    x_r[s, c0 : c0 + c_n, 0 : SR + 1, :],
                                )
                            elif u == NS - 1:
                                nc.sync.dma_start(
                                    st[:, SR + 1 : SR + 2, 1 : W + 1], zp_d.ap()[:c_n, 0:W]
                                )
                                nc.sync.dma_start(
                                    st[:, 0 : SR + 1, 1 : W + 1],
                                    x_r[s, c0 : c0 + c_n, h0 - 1 : H, :],
                                )
                            else:
                                nc.sync.dma_start(
                                    st[:, :, 1 : W + 1],
                                    x_r[s, c0 : c0 + c_n, h0 - 1 : h0 + SR + 1, :],
                                )
                            strips.append((c0, c_n, st))

                        for oc, (o0, o_n) in enumerate(((0, 128), (128, 64))):
                            pc = cpsum.tile([o_n, SR, W], f32, tag=f"pc{oc}")
                            n_acc = KK * 2
                            i = 0
                            for kk in range(KK):
                                kh, kw = divmod(kk, 3)
                                for c0, c_n, st in strips:
                                    lt = lhsT_lo[s] if c0 == 0 else lhsT_hi[s]
                                    nc.tensor.matmul(
                                        pc[:],
                                        lt[:c_n, kk * O + o0 : kk * O + o0 + o_n],
                                        st[:c_n, kh : kh + SR, kw : kw + W],
                                        start=(i == 0),
                                        stop=(i == n_acc - 1),
                                    )
                                    i += 1
                            stg = stgpool.tile([o_n, SR, W], f32, tag=f"stg{oc}")
                            nc.vector.tensor_copy(stg[:], pc[:])
                            nc.sync.dma_start(
                                out_d[s, o0 : o0 + o_n, h0 : h0 + SR, :], stg[:]
                            )

    nc.compile()
    return nc


def _get_compiled():
    global _COMPILED
    if _COMPILED is None:
        _COMPILED = _build()
    return _COMPILED


def kernel(x, expert_weight, routing_w, routing_b, trace=False):
    from concourse.bass_utils import run_bass_kernel_spmd

    nc = _get_compiled()
    ew = np.ascontiguousarray(expert_weight, dtype=np.float32)
    _ZPAD = np.zeros((128, 128), dtype=np.float32)
    rw = np.ascontiguousarray(routing_w, dtype=np.float32)
    rb = np.ascontiguousarray(routing_b, dtype=np.float32)
    in_maps = [
        {
            "x": np.ascontiguousarray(x[i * BS : (i + 1) * BS], dtype=np.float32),
            "x_r": np.ascontiguousarray(x[i * BS : (i + 1) * BS], dtype=np.float32),
            "expert_weight": ew,
            "zpad": _ZPAD,
            "routing_w": rw,
            "routing_b": rb,
        }
        for i in range(N_CORES)
    ]
    res = run_bass_kernel_spmd(
        nc, in_maps, core_ids=list(range(N_CORES)), trace=trace
    )
    out = np.concatenate([res.results[i]["out"] for i in range(N_CORES)], axis=0)
    if trace:
        kernel.last_results = res
    return out


# revision 8
# speedup vs baseline: 203.4144x; 203.4144x over previous
"""CondConv2d Trainium2 kernel.

B=32, C=192, H=W=64, O=192, E=8, 3x3 'same' conv.
Data-parallel over batch: 8 cores x 4 samples. Expert weights replicated.

Per-core pipeline:
  Phase A: stream x, per-(sample,channel) sums (global avg pool numerator).
  Routing: logits = sums @ routing_w.T; sigmoid(logits/4096 + b) on ACT.
  Phase B: mix expert weights with a block-diagonal PE matmul
           (K = 16 o-groups x 8 experts = 128 -> 16x fewer streamed columns
           than the naive K=8 mixing), partition-remap via SBUF->SBUF DMA,
           then PE transposes to per-sample lhsT[c, (kk,o)] layout.
  Phase C: conv = 9 shifted float32r matmuls accumulated in PSUM per
           8-row output strip; evict via DVE; DMA out.
"""

import sys
import numpy as np

for _p in ("/opt/trn_rl_repo",):
    if _p not in sys.path:
        sys.path.insert(0, _p)

BS = 4          # samples per core
C = 192
H = W = 64
O = 192
E = 8
KK = 9          # 3x3
CKK = C * KK    # 1728, per-o flattened (c,kh,kw) block in expert_weight rows
N_CORES = 8

# mixing pass structure: o-groups of 8 o-values
OSUB = 8
G1 = 16         # pass 1: o in [0,128)
G2 = 8          # pass 2: o in [128,192)
GLEN = OSUB * CKK          # 13824 elements of an expert row per group
MIXN = 432                 # mixing matmul free dim (>=256 keeps f32r at 1 cyc/col)
T_PER_G = GLEN // MIXN     # 32

_COMPILED = None


def _build():
    import concourse.bass as bass
    import concourse.bacc as bacc
    import concourse.mybir as mybir
    import concourse.tile as tile
    from concourse import masks

    f32 = mybir.dt.float32
    f32r = mybir.dt.float32r
    AX = mybir.AxisListType
    ACT = mybir.ActivationFunctionType

    nc = bacc.Bacc("TRN2", target_bir_lowering=False, debug=False)

    x_d = nc.dram_tensor("x", [BS, C, H, W], f32, kind="ExternalInput")
    x_r = nc.dram_tensor("x_r", [BS, C, H, W], f32r, kind="ExternalInput")
    ew_d = nc.dram_tensor("expert_weight", [E, O * CKK], f32r, kind="ExternalInput")
    rw_d = nc.dram_tensor("routing_w", [E, C], f32, kind="ExternalInput")
    rb_d = nc.dram_tensor("routing_b", [E], f32, kind="ExternalInput")
    zp_d = nc.dram_tensor("zpad", [128, 128], f32r, kind="ExternalInput")
    out_d = nc.dram_tensor("out", [BS, O, H, W], f32, kind="ExternalOutput")

    with tile.TileContext(nc) as tc:
        with (
            tc.tile_pool(name="persist", bufs=1) as persist,
            tc.tile_pool(name="psum_small", bufs=1, space="PSUM") as psum_small,
        ):
            # ---------- persistent small tiles ----------
            ident = persist.tile([128, 128], f32)
            masks.make_identity(nc, ident[:])

            rwT_lo = persist.tile([128, E], f32)
            rwT_hi = persist.tile([64, E], f32)
            rwT_src = rw_d.ap().rearrange("e c -> c e")
            nc.sync.dma_start(rwT_lo[:], rwT_src[0:128])
            nc.sync.dma_start(rwT_hi[:], rwT_src[128:192])
            rb_t = persist.tile([E, 1], f32)
            nc.sync.dma_start(rb_t[:], rb_d.ap().unsqueeze(1))

            pooled_lo = persist.tile([128, BS], f32)
            pooled_hi = persist.tile([64, BS], f32)

            # per-sample conv weights, transposed layout [c, kk*192+o]
            lhsT_lo = [persist.tile([128, KK * O], f32r, name=f"lhsT_lo{b}") for b in range(BS)]
            lhsT_hi = [persist.tile([64, KK * O], f32r, name=f"lhsT_hi{b}") for b in range(BS)]

            # ---------- Phase A: x sums for global avg pool ----------
            with tc.tile_pool(name="apool", bufs=2) as apool:
                for s in range(BS):
                    xa_lo = apool.tile([128, H, W], f32, tag="xa_lo")
                    nc.sync.dma_start(xa_lo[:], x_d[s, 0:128])
                    nc.vector.reduce_sum(pooled_lo[:, s : s + 1], xa_lo[:], axis=AX.XY)
                    xa_hi = apool.tile([64, H, W], f32, tag="xa_hi")
                    nc.sync.dma_start(xa_hi[:], x_d[s, 128:192])
                    nc.vector.reduce_sum(pooled_hi[:, s : s + 1], xa_hi[:], axis=AX.XY)

            # ---------- routing ----------
            psum_r = psum_small.tile([E, BS], f32)
            nc.tensor.matmul(psum_r[:], rwT_lo[:], pooled_lo[:], start=True, stop=False)
            nc.tensor.matmul(psum_r[:], rwT_hi[:], pooled_hi[:], start=False, stop=True)
            r_sb = persist.tile([E, BS], f32r)
            nc.scalar.activation(
                r_sb[:], psum_r[:], ACT.Sigmoid, bias=rb_t[:], scale=1.0 / (H * W)
            )

            # block-diagonal mixing weights bd[(g,e), (g,b)] = r[b,e]
            bd1 = persist.tile([G1 * E, G1 * BS], f32r)
            bd2 = persist.tile([G2 * E, G2 * BS], f32r)
            nc.sync.dma_start(bd1[:], zp_d.ap()[: G1 * E, : G1 * BS])
            nc.sync.dma_start(bd2[:], zp_d.ap()[: G2 * E, : G2 * BS])
            for g in range(G1):
                nc.sync.dma_start(
                    bd1[g * E : (g + 1) * E, g * BS : (g + 1) * BS], r_sb[:]
                )
            for g in range(G2):
                nc.sync.dma_start(
                    bd2[g * E : (g + 1) * E, g * BS : (g + 1) * BS], r_sb[:]
                )

            # ---------- Phase B: mix + remap + transpose ----------
            with (
                tc.tile_pool(name="spool", bufs=1) as spool,
                tc.tile_pool(name="natpool", bufs=1) as natpool,
                tc.tile_pool(name="epool", bufs=8) as epool,
                tc.tile_pool(name="mixpsum", bufs=3, space="PSUM") as mixpsum,
                tc.tile_pool(name="tpsum", bufs=3, space="PSUM") as tpsum,
            ):
                nat_lo = [natpool.tile([128, CKK], f32, name=f"nat_lo{b}") for b in range(BS)]
                nat_hi = [natpool.tile([64, CKK], f32, name=f"nat_hi{b}") for b in range(BS)]

                for pidx, (G, bd, nat, o_base) in enumerate(
                    ((G1, bd1, nat_lo, 0), (G2, bd2, nat_hi, 128))
                ):
                    S_all = spool.tile([G * BS, GLEN], f32, tag="S_all", name=f"S{pidx}")
                    ew_view = ew_d.ap()[:, o_base * CKK : (o_base + G * OSUB) * CKK]
                    ew_view = ew_view.rearrange("e (g u) -> g e u", g=G)
                    for t in range(T_PER_G):
                        ewt = epool.tile([G * E, MIXN], f32r, tag="ewt")
                        nc.sync.dma_start(ewt[:], ew_view[:, :, t * MIXN : (t + 1) * MIXN])
                        pm = mixpsum.tile([G * BS, MIXN], f32, tag="pm")
                        nc.tensor.matmul(
                            pm[:], bd[:], ewt[:],
                            start=True, stop=True,
                        )
                        nc.vector.tensor_copy(S_all[:, t * MIXN : (t + 1) * MIXN], pm[:])
                    # partition remap: row (g*BS+b) cols (o_sub,c,kk) -> nat[b][o, (c,kk)]
                    for b in range(BS):
                        for g in range(G):
                            nc.sync.dma_start(
                                nat[b][g * OSUB : (g + 1) * OSUB, :],
                                S_all[g * BS + b : g * BS + b + 1, :].rearrange(
                                    "p (o u) -> p o u", o=OSUB
                                ),
                            )

                # transposes: nat[b][o, (c,kk)] -> lhsT[b][c, (kk,o)]
                for b in range(BS):
                    for kk in range(KK):
                        for cc, (c0, c_n, lhsT) in enumerate(
                            ((0, 128, lhsT_lo[b]), (128, 64, lhsT_hi[b]))
                        ):
                            for oc, (o0, o_n, nat) in enumerate(
                                ((0, 128, nat_lo[b]), (128, 64, nat_hi[b]))
                            ):
                                src = nat[:].rearrange("o (c k) -> o c k", k=KK)[
                                    :, c0 : c0 + c_n, kk
                                ]
                                tp = tpsum.tile([128, 128], f32, tag="tp")
                                nc.tensor.transpose(
                                    tp[:c_n, :o_n], src, ident[:o_n, :o_n]
                                )
                                nc.scalar.copy(
                                    lhsT[:c_n, kk * O + o0 : kk * O + o0 + o_n],
                                    tp[:c_n, :o_n],
                                )

            # ---------- Phase C: conv ----------
            NS = 8          # row strips per sample
            SR = H // NS    # 8 output rows per strip
            with (
                tc.tile_pool(name="cpool", bufs=3) as cpool,
                tc.tile_pool(name="stgpool", bufs=3) as stgpool,
                tc.tile_pool(name="cpsum", bufs=3, space="PSUM") as cpsum,
            ):
                for s in range(BS):
                    for u in range(NS):
                        h0 = u * SR
                        strips = []
                        for cc, (c0, c_n) in enumerate(((0, 128), (128, 64))):
                            st = cpool.tile([c_n, SR + 2, W + 2], f32r, tag=f"strip{cc}")
                            nc.sync.dma_start(st[:, :, 0:1], zp_d.ap()[:c_n, 0 : SR + 2])
                            nc.sync.dma_start(st[:, :, W + 1 : W + 2], zp_d.ap()[:c_n, 0 : SR + 2])
                            if u == 0:
                                nc.sync.dma_start(st[:, 0:1, 1 : W + 1], zp_d.ap()[:c_n, 0:W])
                                nc.sync.dma_start(
                                    st[:, 1 : SR + 2, 1 : W + 1],
                                    x_r[s, c0 : c0 + c_n, 0 : SR + 1, :],
                                )
                            elif u == NS - 1:
                                nc.sync.dma_start(
                                    st[:, SR + 1 : SR + 2, 1 : W + 1], zp_d.ap()[:c_n, 0:W]
                                )
                                nc.sync.dma_start(
                                    st[:, 0 : SR + 1, 1 : W + 1],
                                    x_r[s, c0 : c0 + c_n, h0 - 1 : H, :],
                                )
                            else:
                                nc.sync.dma_start(
                                    st[:, :, 1 : W + 1],
                                    x_r[s, c0 : c0 + c_n, h0 - 1 : h0 + SR + 1, :],
                                )
                            strips.append((c0, c_n, st))

                        for oc, (o0, o_n) in enumerate(((0, 128), (128, 64))):
                            pc = cpsum.tile([o_n, SR, W], f32, tag=f"pc{oc}")
                            n_acc = KK * 2
                            i = 0
                            for kk in range(KK):
                                kh, kw = divmod(kk, 3)
                                for c0, c_n, st in strips:
                                    lt = lhsT_lo[s] if c0 == 0 else lhsT_hi[s]
                                    nc.tensor.matmul(
                                        pc[:],
                                        lt[:c_n, kk * O + o0 : kk * O + o0 + o_n],
                                        st[:c_n, kh : kh + SR, kw : kw + W],
                                        start=(i == 0),
                                        stop=(i == n_acc - 1),
                                    )
                                    i += 1
                            stg = stgpool.tile([o_n, SR, W], f32, tag=f"stg{oc}")
                            nc.vector.tensor_copy(stg[:], pc[:])
                            nc.sync.dma_start(
                                out_d[s, o0 : o0 + o_n, h0 : h0 + SR, :], stg[:]
                            )

    nc.compile()
    return nc


def _get_compiled():
    global _COMPILED
    if _COMPILED is None:
        _COMPILED = _build()
    return _COMPILED


def kernel(x, expert_weight, routing_w, routing_b, trace=False):
    from concourse.bass_utils import run_bass_kernel_spmd

    nc = _get_compiled()
    ew = np.ascontiguousarray(expert_weight, dtype=np.float32)
    _ZPAD = np.zeros((128, 128), dtype=np.float32)
    rw = np.ascontiguousarray(routing_w, dtype=np.float32)
    rb = np.ascontiguousarray(routing_b, dtype=np.float32)
    in_maps = [
        {
            "x": np.ascontiguousarray(x[i * BS : (i + 1) * BS], dtype=np.float32),
            "x_r": np.ascontiguousarray(x[i * BS : (i + 1) * BS], dtype=np.float32),
            "expert_weight": ew,
            "zpad": _ZPAD,
            "routing_w": rw,
            "routing_b": rb,
        }
        for i in range(N_CORES)
    ]
    res = run_bass_kernel_spmd(
        nc, in_maps, core_ids=list(range(N_CORES)), trace=trace
    )
    out = np.concatenate([res.results[i]["out"] for i in range(N_CORES)], axis=0)
    if trace:
        kernel.last_results = res
    return out
